# revision 13
# baseline (speedup 1.0000x reference)
"""Cross-attention kernel for Trainium2, 8 NeuronCores.

Problem (full shapes): B=4, Lq=Lk=2048, D(in)=D(out)=1024, fp32.
    q = query @ Wq + bq ; k = key @ Wk + bk ; v = value @ Wv + bv
    out = softmax(q k^T / sqrt(1024)) v

Sharding: 8 cores = (batch b, half h of Lq). Each core computes a
[1024, 1024] slice of the output for batch b, query rows
[h*1024, (h+1)*1024). K/V projections for a batch are duplicated across
the core pair (cheaper than cross-core exchange for this size).

Per-core layouts (P=128 partitions):
  - Projections are computed with the contraction dim (d) on partitions;
    host supplies q/k/v inputs pre-transposed ([D, L]).
  - qT [do, lq] and kT [do, lk] are produced feature-major, v [lk, do]
    natural.  Scores are computed transposed, ST[lk, lq], so the whole
    attention pipeline needs no on-device transposes.
  - Scores are small (|S/32| < ~3) so exp is applied without
    max-subtraction; row sums r[lq] come from a PE matmul with a ones
    column; normalization is a per-partition scalar multiply on the
    natural-layout output.  v carries its bias, so softmax rows summing
    to 1 makes the output bias exact with no extra broadcast add.
"""

import os
import sys

sys.path.insert(0, "/opt/trn_rl_repo")

from contextlib import ExitStack

import numpy as np

import concourse.bass as bass
import concourse.tile as tile
from concourse import bacc, mybir
from concourse.bass_utils import run_bass_kernel_spmd

P = 128
B, LQ, LK, D = 4, 2048, 2048, 1024
NCORES = 8
LQS = LQ * B // NCORES  # 1024 query rows per core
KCH = D // P  # 8 contraction chunks
DOT = D // P  # 8 output-feature tiles
LKT = LK // P  # 16 key tiles
PBLK = 512  # projection matmul free dim
ABLK = 256  # attention lq block (2 lq tiles)
SCALE = 1.0 / 32.0  # 1/sqrt(D)

F32 = mybir.dt.float32

# Matmul dtype mode: "f32" (exact, 4 cyc/row), "f32r" (fp32 storage
# rounded to a ~12-bit mantissa, single-pass matmul at bf16 rate, 1
# cyc/row for free dim >= 256), "bf16".
MM_MODE = os.environ.get("KMODE", "f32r")


def _mm_dtypes(mode):
    if mode == "bf16":
        import ml_dtypes

        return mybir.dt.bfloat16, np.dtype(ml_dtypes.bfloat16)
    if mode == "f32r":
        return mybir.dt.float32r, np.dtype(np.float32)
    return F32, np.dtype(np.float32)


DT, HOST_DT = _mm_dtypes(MM_MODE)


def _round_host(a):
    """Round fp32 host data the way the hardware rounds float32r
    producers (round-to-nearest-even keeping 11 mantissa bits), so
    DMA-fed f32r operands match what on-device rounding would give."""
    if MM_MODE != "f32r":
        return np.ascontiguousarray(a)
    b = np.ascontiguousarray(a).view(np.uint32).astype(np.uint64)
    keep = 12  # bits dropped
    mask = (np.uint64(0xFFFFFFFF) >> np.uint64(keep)) << np.uint64(keep)
    half = np.uint64(1) << np.uint64(keep - 1)
    low = b & ~mask & np.uint64(0xFFFFFFFF)
    rup = (b + half) & mask
    tie = low == half
    lsb = (b >> np.uint64(keep)) & np.uint64(1)
    out = rup
    down = tie & (lsb == 0)
    out[down] = (b & mask)[down]
    return out.astype(np.uint32).view(np.float32)


def build_program(repeat=1):
    nc = bacc.Bacc("TRN2", target_bir_lowering=False, debug=False)

    q_inT = nc.dram_tensor("q_inT", [D, LQS], DT, kind="ExternalInput").ap()
    k_inT = nc.dram_tensor("k_inT", [D, LK], DT, kind="ExternalInput").ap()
    v_inT = nc.dram_tensor("v_inT", [D, LK], DT, kind="ExternalInput").ap()
    Wq = nc.dram_tensor("Wq", [D, D], DT, kind="ExternalInput").ap()
    Wk = nc.dram_tensor("Wk", [D, D], DT, kind="ExternalInput").ap()
    Wv = nc.dram_tensor("Wv", [D, D], DT, kind="ExternalInput").ap()
    bq = nc.dram_tensor("bq", [D], F32, kind="ExternalInput").ap()
    bk = nc.dram_tensor("bk", [D], F32, kind="ExternalInput").ap()
    bv = nc.dram_tensor("bv", [D], F32, kind="ExternalInput").ap()
    out = nc.dram_tensor("out", [LQS, D], F32, kind="ExternalOutput").ap()

    q_inT_t = q_inT.rearrange("(o p) l -> p o l", p=P)
    k_inT_t = k_inT.rearrange("(o p) l -> p o l", p=P)
    v_inT_t = v_inT.rearrange("(o p) l -> p o l", p=P)
    Wq_t = Wq.rearrange("(o p) n -> p o n", p=P)
    Wk_t = Wk.rearrange("(o p) n -> p o n", p=P)
    Wv_t = Wv.rearrange("(o p) n -> p o n", p=P)

    with tile.TileContext(nc) as tc, ExitStack() as ctx:
        psum_mm = ctx.enter_context(tc.tile_pool(name="psum_mm", bufs=4, space="PSUM"))
        psum_st = ctx.enter_context(tc.tile_pool(name="psum_st", bufs=2, space="PSUM"))
        psum_r = ctx.enter_context(tc.tile_pool(name="psum_r", bufs=2, space="PSUM"))
        singles = ctx.enter_context(tc.tile_pool(name="singles", bufs=1))
        kt_pool = ctx.enter_context(tc.tile_pool(name="kt_pool", bufs=1))
        v_pool = ctx.enter_context(tc.tile_pool(name="v_pool", bufs=1))
        dram = ctx.enter_context(tc.tile_pool(name="dram", bufs=1, space="DRAM"))
        tc._pt_pool = ctx.enter_context(tc.tile_pool(name="pt_pool", bufs=6))

        # ---- constants -------------------------------------------------
        bq_sb = singles.tile([P, DOT], F32, name="bq_sb")
        nc.sync.dma_start(bq_sb[:], bq.rearrange("(o p) -> p o", p=P))
        bk_sb = singles.tile([P, DOT], F32, name="bk_sb")
        nc.sync.dma_start(bk_sb[:], bk.rearrange("(o p) -> p o", p=P))
        # bv broadcast to all 128 partitions (stride-0 partition read)
        bv_rep = singles.tile([P, D], F32, name="bv_rep")
        bv_bcast = bass.AP(tensor=bv.tensor, offset=bv.offset, ap=[[0, P], *bv.ap])
        nc.gpsimd.dma_start(bv_rep[:], bv_bcast)
        if MM_MODE == "f32r":
            ones_f = singles.tile([P, 2], F32, name="ones_f")
            nc.vector.memset(ones_f[:], 1.0)
            ones_sb = singles.tile([P, 2], DT, name="ones_sb")
            nc.vector.tensor_scalar_add(ones_sb[:], ones_f[:], 0.0)
        else:
            ones_sb = singles.tile([P, 2], DT, name="ones_sb")
            nc.vector.memset(ones_sb[:], 1.0)

        phases = os.environ.get("KPHASES", "all")
        for _rep in range(repeat):
            one_pass(nc, tc, singles, kt_pool, v_pool, dram, psum_mm, psum_st,
                     psum_r, bq_sb, bk_sb, bv_rep, ones_sb,
                     q_inT_t, k_inT_t, v_inT_t, Wq_t, Wk_t, Wv_t, out,
                     phases=phases)

    nc.compile()
    return nc


def pt_pool_tile(tc, name):
    return tc._pt_pool.tile([P, ABLK], DT, tag="pt", name=name)


def one_pass(nc, tc, singles, kt_pool, v_pool, dram, psum_mm, psum_st, psum_r,
             bq_sb, bk_sb, bv_rep, ones_sb,
             q_inT_t, k_inT_t, v_inT_t, Wq_t, Wk_t, Wv_t, out, phases="all"):
    do_proj = phases in ("all", "proj")
    do_attn = phases in ("all", "attn")
    with tc.tile_pool(name="wpool", bufs=1) as wpool:
        kT_sb = kt_pool.tile([P, DOT, LK], DT, tag="kT_sb", name="kT_sb")
        v_sb = v_pool.tile([P, LKT, D], DT, tag="v_sb", name="v_sb")

        # ---- phase 0: kT = (k_in @ Wk + bk)^T, SBUF-resident ----------
        if not do_proj:
            nc.vector.memset(kT_sb[:, 0, 0:2].bitcast(F32), 0.001)
            nc.vector.memset(v_sb[:, 0, 0:2].bitcast(F32), 0.001)
        if do_proj:
          with tc.tile_pool(name="kin_pool", bufs=2) as kin_pool:
            Wk_sb = wpool.tile([P, KCH, D], DT, tag="W", name="Wk_sb")
            for o in range(KCH):
                nc.sync.dma_start(Wk_sb[:, o], Wk_t[:, o])
            for n in range(LK // PBLK):
                kin = kin_pool.tile([P, KCH, PBLK], DT, tag="kin", name="kin")
                for o in range(KCH):
                    eng = nc.sync if o % 2 == 0 else nc.scalar
                    eng.dma_start(
                        kin[:, o], k_inT_t[:, o, n * PBLK : (n + 1) * PBLK]
                    )
                for m in range(DOT):
                    ps = psum_mm.tile([P, PBLK], F32, tag="mm", name="ps_k")
                    for k in range(KCH):
                        nc.tensor.matmul(
                            ps[:],
                            Wk_sb[:, k, m * P : (m + 1) * P],
                            kin[:, k],
                            start=(k == 0),
                            stop=(k == KCH - 1),
                        )
                    nc.vector.tensor_scalar_add(
                        kT_sb[:, m, n * PBLK : (n + 1) * PBLK],
                        ps[:],
                        bk_sb[:, m : m + 1],
                    )

        # ---- phase 1: v = v_in @ Wv + bv, natural layout, resident ----
        if do_proj:
          with tc.tile_pool(name="vin_pool", bufs=2) as vin_pool:
            Wv_sb = wpool.tile([P, KCH, D], DT, tag="W", name="Wv_sb")
            for o in range(KCH):
                nc.sync.dma_start(Wv_sb[:, o], Wv_t[:, o])
            for blk in range(LK // PBLK):  # 4 blocks of 512 key rows
                vin = vin_pool.tile([P, KCH, PBLK], DT, tag="vin", name="vin")
                for o in range(KCH):
                    eng = nc.sync if o % 2 == 0 else nc.scalar
                    eng.dma_start(
                        vin[:, o], v_inT_t[:, o, blk * PBLK : (blk + 1) * PBLK]
                    )
                for t in range(PBLK // P):
                    lk_t = blk * (PBLK // P) + t
                    for dh in range(D // PBLK):
                        ps = psum_mm.tile([P, PBLK], F32, tag="mm", name="ps_v")
                        for k in range(KCH):
                            nc.tensor.matmul(
                                ps[:],
                                vin[:, k, t * P : (t + 1) * P],
                                Wv_sb[:, k, dh * PBLK : (dh + 1) * PBLK],
                                start=(k == 0),
                                stop=(k == KCH - 1),
                            )
                        nc.vector.tensor_add(
                            v_sb[:, lk_t, dh * PBLK : (dh + 1) * PBLK],
                            ps[:],
                            bv_rep[:, dh * PBLK : (dh + 1) * PBLK],
                        )

        # ---- phase 2: attention with fused q projection ---------------
        if not do_attn:
            return
        Wq_sb = wpool.tile([P, KCH, D], DT, tag="W", name="Wq_sb")
        for o in range(KCH):
            nc.sync.dma_start(Wq_sb[:, o], Wq_t[:, o])
        with tc.tile_pool(name="qin_pool", bufs=2) as qin_pool, tc.tile_pool(
            name="qtb_pool", bufs=2
        ) as qtb_pool, tc.tile_pool(name="osb_pool", bufs=2) as osb_pool, tc.tile_pool(
            name="rsb_pool", bufs=2
        ) as rsb_pool:
            for blk in range(LQS // ABLK):  # 4 blocks of 256 query rows
                qin = qin_pool.tile([P, KCH, ABLK], DT, tag="qin", name="qin")
                for o in range(KCH):
                    eng = nc.sync if o % 2 == 0 else nc.scalar
                    eng.dma_start(
                        qin[:, o], q_inT_t[:, o, blk * ABLK : (blk + 1) * ABLK]
                    )
                # q projection for this block: qtb[do, lq]
                qtb = qtb_pool.tile([P, KCH, ABLK], DT, tag="qtb", name="qtb")
                for m in range(DOT):
                    ps = psum_st.tile([P, ABLK], F32, tag="st", name="ps_qp")
                    for k in range(KCH):
                        nc.tensor.matmul(
                            ps[:],
                            Wq_sb[:, k, m * P : (m + 1) * P],
                            qin[:, k],
                            start=(k == 0),
                            stop=(k == KCH - 1),
                        )
                    nc.vector.tensor_scalar_add(
                        qtb[:, m], ps[:], bq_sb[:, m : m + 1]
                    )
                o_ps = [
                    [
                        psum_mm.tile([P, PBLK], F32, tag="mm", name=f"o_ps_{t}_{dh}")
                        for dh in range(D // PBLK)
                    ]
                    for t in range(ABLK // P)
                ]
                r_ps = [
                    psum_r.tile([P, 2], F32, tag="r", name=f"r_ps_{t}")
                    for t in range(ABLK // P)
                ]
                for c in range(LKT):
                    st = psum_st.tile([P, ABLK], F32, tag="st", name="st")
                    for o in range(KCH):
                        nc.tensor.matmul(
                            st[:],
                            kT_sb[:, o, c * P : (c + 1) * P],
                            qtb[:, o],
                            start=(o == 0),
                            stop=(o == KCH - 1),
                        )
                    pt = pt_pool_tile(tc, name="pt")
                    nc.scalar.activation(
                        pt[:], st[:], mybir.ActivationFunctionType.Exp, scale=SCALE
                    )
                    for t in range(ABLK // P):
                        pt_t = pt[:, t * P : (t + 1) * P]
                        for dh in range(D // PBLK):
                            nc.tensor.matmul(
                                o_ps[t][dh][:],
                                pt_t,
                                v_sb[:, c, dh * PBLK : (dh + 1) * PBLK],
                                start=(c == 0),
                                stop=(c == LKT - 1),
                            )
                        nc.tensor.matmul(
                            r_ps[t][:],
                            pt_t,
                            ones_sb[:],
                            start=(c == 0),
                            stop=(c == LKT - 1),
                        )
                for t in range(ABLK // P):
                    rsb = rsb_pool.tile([P, 1], F32, tag="rsb", name="rsb")
                    nc.vector.reciprocal(rsb[:], r_ps[t][:, 0:1])
                    lq0 = blk * ABLK + t * P
                    for dh in range(D // PBLK):
                        osb = osb_pool.tile([P, PBLK], F32, tag="osb", name="osb")
                        nc.scalar.mul(osb[:], o_ps[t][dh][:], rsb[:])
                        nc.sync.dma_start(
                            out[lq0 : lq0 + P, dh * PBLK : (dh + 1) * PBLK], osb[:]
                        )


_program = None


def _get_program():
    global _program
    if _program is None:
        _program = build_program()
    return _program


def _make_in_maps(query_input, key_input, value_input, Wq, bq, Wk, bk, Wv, bv):
    f32 = np.float32
    Wq_h = _round_host(np.asarray(Wq, HOST_DT))
    Wk_h = _round_host(np.asarray(Wk, HOST_DT))
    Wv_h = _round_host(np.asarray(Wv, HOST_DT))
    bq_h = np.asarray(bq, f32)
    bk_h = np.asarray(bk, f32)
    bv_h = np.asarray(bv, f32)
    in_maps = []
    kT_cache = {}
    for c in range(NCORES):
        b, h = divmod(c, 2)
        if b not in kT_cache:
            kT_cache[b] = (
                _round_host(np.asarray(key_input[b], HOST_DT).T),
                _round_host(np.asarray(value_input[b], HOST_DT).T),
            )
        k_t, v_t = kT_cache[b]
        q_sh = np.asarray(query_input[b, h * LQS : (h + 1) * LQS, :], HOST_DT)
        in_maps.append(
            {
                "q_inT": _round_host(q_sh.T),
                "k_inT": k_t,
                "v_inT": v_t,
                "Wq": Wq_h,
                "Wk": Wk_h,
                "Wv": Wv_h,
                "bq": bq_h,
                "bk": bk_h,
                "bv": bv_h,
            }
        )
    return in_maps


def run(in_maps, **kwargs):
    nc = _get_program()
    return run_bass_kernel_spmd(nc, in_maps, core_ids=list(range(NCORES)), **kwargs)


def kernel(query_input, key_input, value_input, Wq, bq, Wk, bk, Wv, bv):
    in_maps = _make_in_maps(
        query_input, key_input, value_input, Wq, bq, Wk, bk, Wv, bv
    )
    res = run(in_maps)
    out = np.empty((B, LQ, D), np.float32)
    for c in range(NCORES):
        b, h = divmod(c, 2)
        out[b, h * LQS : (h + 1) * LQS, :] = res.results[c]["out"]
    return out
